# revision 1
# baseline (speedup 1.0000x reference)
"""ContextualAttentionMask Trainium2 kernel.

Math (per batch sample):
  f: [256, 4096] feature map (channels x pixels), m: [4096] mask
  K[j, :]    = f[:, j] + 1e-7          (per-pixel 1x1 kernel)
  rstd[j]    = 1 / ||K[j, :]||_2
  raw[j, n]  = sum_c f[c, j] * f[c, n]          (only interior columns matter:
               the conv padding columns are dead compute - 1x1 kernels, the
               output at pad positions is cropped, softmax is per-column)
  att[j, n]  = softmax_j(rstd[j] * raw[j, n])
  fmap[c, n] = sum_j rstd[j] * m[j] * K[j, c] * att[j, n]
  final      = fmap * (1 - m) + f * m  ;  skip branch if mask nearly all-ones

Device computes (per core, unnormalized; host divides, blends, skip-branch):
  E[j, n] = exp(rstd[j] * raw[j, n] - 12)       (-12 keeps E in fp16 range;
                                                 cancels in the division)
  o[c, n] = sum_j km16[j, c] * E[j, n]     with km16 = fp16(rstd * m * K)
  s[n]    = sum_j E[j, n]

Sharding: 8 cores = 4 samples x 2 column-halves (2048 columns each).
Inputs are host-permuted so each core's own half is always columns 0..2047;
the j (softmax/contraction) order is irrelevant as long as f16/km16/rstd
agree. Tiny per-j scalars (rstd, rstd*m) and fp16 casts are host-side prep;
all heavy compute (2x 2048x4096x256 GEMMs + softmax) runs on device.
"""

import sys
from contextlib import ExitStack

import numpy as np

sys.path.insert(0, "/opt/trn_rl_repo")

from concourse import bacc, mybir, tile  # noqa: E402
from concourse.bass_utils import run_bass_kernel_spmd  # noqa: E402

FP32 = mybir.dt.float32
FP16 = mybir.dt.float16

CH = 256          # channels
J = 4096          # number of per-pixel kernels (= h*w)
NH = 2048         # columns handled per core (half of a sample)
EXP_BIAS = -12.0  # exp(x - 12) keeps values in fp16 range; cancels on host


def build_program(ch=CH, j_total=J, n_half=NH, bufs_sc=5, bufs_out=3,
                  bufs_e=13, loop_reps=1):
    """Emit the per-core Bass/Tile program (SPMD across 8 cores)."""
    assert ch % 128 == 0 and j_total % 128 == 0
    n_cb = ch // 128          # channel blocks
    n_jb = j_total // 128     # j blocks
    qs = min(512, n_half)     # output column chunk width
    nq = n_half // qs
    assert n_half % qs == 0

    nc = bacc.Bacc("TRN2", target_bir_lowering=False, debug=False, num_devices=8)

    f_d = nc.dram_tensor("f16", [ch, j_total], FP16, kind="ExternalInput").ap()
    km_d = nc.dram_tensor("km16", [j_total, ch], FP16, kind="ExternalInput").ap()
    rstd_d = nc.dram_tensor("rstd", [128, n_jb], FP32, kind="ExternalInput").ap()
    o_d = nc.dram_tensor("o", [ch, n_half], FP32, kind="ExternalOutput").ap()
    s_d = nc.dram_tensor("s", [1, n_half], FP32, kind="ExternalOutput").ap()

    with tile.TileContext(nc) as tc, ExitStack() as ctx:
        const_p = ctx.enter_context(tc.tile_pool(name="const", bufs=1))
        kt_p = ctx.enter_context(tc.tile_pool(name="kt", bufs=n_cb))
        km_p = ctx.enter_context(tc.tile_pool(name="km", bufs=n_jb))
        e_p = ctx.enter_context(tc.tile_pool(name="e", bufs=bufs_e))
        osb_p = ctx.enter_context(tc.tile_pool(name="osb", bufs=3))
        ssb_p = ctx.enter_context(tc.tile_pool(name="ssb", bufs=2))
        ps_sc = ctx.enter_context(
            tc.tile_pool(name="ps_sc", bufs=bufs_sc, space="PSUM"))
        ps_out = ctx.enter_context(
            tc.tile_pool(name="ps_out", bufs=bufs_out, space="PSUM"))

        ones32 = const_p.tile([128, 1], FP32)
        nc.vector.memset(ones32[:], 1.0)
        bias_e = const_p.tile([128, 1], FP32, tag="bias_e")
        nc.vector.memset(bias_e[:], EXP_BIAS)
        rstd = const_p.tile([128, n_jb], FP32, tag="rstd")

        # fp16 feature map, [c, j] layout; chunked DMA so matmuls start early.
        # The small rstd transfer rides after the first chunk pair: early
        # enough for the first exp, without delaying the first matmuls.
        kt = [
            kt_p.tile([128, j_total], FP16, tag="kt", name=f"kt{cb}")
            for cb in range(n_cb)
        ]
        if j_total >= 4096:  # small first chunks so the first matmuls start early
            bounds = [0, 512, 1024, 2048, j_total]
        else:
            bounds = list(range(0, j_total + 1, min(512, j_total)))
        for i, (q8, q9) in enumerate(zip(bounds[:-1], bounds[1:])):
            for cb in range(n_cb):
                nc.sync.dma_start(
                    out=kt[cb][:, q8:q9],
                    in_=f_d[cb * 128:(cb + 1) * 128, q8:q9],
                )
            if i == 0:
                nc.sync.dma_start(out=rstd[:], in_=rstd_d[:, :])

        # mask-and-norm-scaled kernels, [j, c] layout
        km = []
        for jb in range(n_jb):
            t = km_p.tile([128, ch], FP16, tag="km", name=f"km{jb}")
            nc.sync.dma_start(out=t[:], in_=km_d[jb * 128:(jb + 1) * 128, :])
            km.append(t)

        # fused main loop: scores -> exp -> sumexp & Km^T E accumulation.
        # The softmax denominator is folded partition-wise on the (idle) DVE
        # (acc[p, n] = sum_jb E[jb*128+p, n]); one fp32 ones-matmul per chunk
        # does the final 128-way fold, keeping the PE stream count minimal.
        # loop_reps > 1 repeats the identical work (timing experiments only).
        for q in [qq for _ in range(loop_reps) for qq in range(nq)]:
            nsl = slice(q * qs, (q + 1) * qs)
            sum_ps = ps_out.tile([1, qs], FP32, tag="out", name="sum_ps")
            acc = ssb_p.tile([128, qs], FP32, tag="acc", name="acc")
            out_ps = [
                ps_out.tile([128, qs], FP32, tag="out", name=f"out_ps{cb}")
                for cb in range(n_cb)
            ]
            # software pipeline: the exp-dependent matmuls trail the score
            # matmuls by D j-blocks, so the in-order PE queue never waits on
            # the ACT exp latency (recovers ~6 us of 117 ns/jb stalls).
            D = min(3, n_jb - 1)
            etiles = {}
            for jj in range(n_jb + D):
                if jj < n_jb:
                    jb = jj
                    jsl = slice(jb * 128, (jb + 1) * 128)
                    ps = ps_sc.tile([128, qs], FP32, tag="sc", name="ps")
                    for cb in range(n_cb):
                        nc.tensor.matmul(
                            ps[:], kt[cb][:, jsl], kt[cb][:, nsl],
                            start=(cb == 0), stop=(cb == n_cb - 1),
                        )
                    e = e_p.tile([128, qs], FP16, tag="e", name="e")
                    nc.scalar.activation(
                        e[:], ps[:], mybir.ActivationFunctionType.Exp,
                        bias=bias_e[:], scale=rstd[:, jb:jb + 1],
                    )
                    etiles[jb] = e
                if jj >= D:
                    jb = jj - D
                    e = etiles.pop(jb)
                    if jb == 0:
                        nc.vector.tensor_copy(acc[:], e[:])
                    else:
                        nc.vector.tensor_add(acc[:], acc[:], e[:])
                    for cb in range(n_cb):
                        nc.tensor.matmul(
                            out_ps[cb][:], km[jb][:, cb * 128:(cb + 1) * 128], e[:],
                            start=(jb == 0), stop=(jb == n_jb - 1),
                        )
            nc.tensor.matmul(sum_ps[:], ones32[:], acc[:], start=True, stop=True)
            srow = ssb_p.tile([1, qs], FP32, tag="srow", name="srow")
            nc.vector.tensor_copy(srow[:], sum_ps[:])
            nc.sync.dma_start(out=s_d[0:1, nsl], in_=srow[:])
            for cb in range(n_cb):
                osb = osb_p.tile([128, qs], FP32, tag="osb", name="osb")
                nc.vector.tensor_copy(osb[:], out_ps[cb][:])
                nc.sync.dma_start(out=o_d[cb * 128:(cb + 1) * 128, nsl], in_=osb[:])

    nc.compile()
    return nc


_CACHE = {}


def _get_program():
    if "nc" not in _CACHE:
        _CACHE["nc"] = build_program()
    return _CACHE["nc"]


def _get_runner():
    """Cached sharded executable over 8 cores (same program/plugin as
    run_bass_kernel_spmd's axon path, but without per-call retracing)."""
    if "runner" in _CACHE:
        return _CACHE["runner"]
    import jax
    from jax.sharding import Mesh, NamedSharding, PartitionSpec
    from jax.experimental.shard_map import shard_map
    from concourse import bass2jax, mybir
    from concourse.bass2jax import _bass_exec_p, partition_id_tensor

    nc = _get_program()
    bass2jax.install_neuronx_cc_hook()
    pname = nc.partition_id_tensor.name if nc.partition_id_tensor else None

    in_names, out_names, out_avals = [], [], []
    for alloc in nc.m.functions[0].allocations:
        if not isinstance(alloc, mybir.MemoryLocationSet):
            continue
        name = alloc.memorylocations[0].name
        if alloc.kind == "ExternalInput":
            if name != pname:
                in_names.append(name)
        elif alloc.kind == "ExternalOutput":
            out_names.append(name)
            out_avals.append(
                jax.core.ShapedArray(
                    tuple(alloc.tensor_shape), mybir.dt.np(alloc.dtype)
                )
            )
    n_params, n_outs = len(in_names), len(out_names)
    all_in = in_names + out_names + ([pname] if pname else [])

    def _body(*args):
        operands = list(args)
        if pname is not None:
            operands.append(partition_id_tensor())
        return tuple(_bass_exec_p.bind(
            *operands, out_avals=tuple(out_avals), in_names=tuple(all_in),
            out_names=tuple(out_names), lowering_input_output_aliases=(),
            sim_require_finite=True, sim_require_nnan=True, nc=nc,
        ))

    devices = jax.devices()[:8]
    mesh = Mesh(np.asarray(devices), ("core",))
    spec = NamedSharding(mesh, PartitionSpec("core"))
    fn = jax.jit(
        shard_map(
            _body, mesh=mesh,
            in_specs=(PartitionSpec("core"),) * (n_params + n_outs),
            out_specs=(PartitionSpec("core"),) * n_outs,
            check_rep=False,
        ),
        donate_argnums=tuple(range(n_params, n_params + n_outs)),
        keep_unused=True,
    )
    zero_host = [
        np.zeros((8 * a.shape[0], *a.shape[1:]), a.dtype) for a in out_avals
    ]

    def run(in_maps):
        concat_in = [
            np.concatenate([np.asarray(m[name]) for m in in_maps], axis=0)
            for name in in_names
        ]
        zeros = [jax.device_put(z, spec) for z in zero_host]
        out = fn(*concat_in, *zeros)
        return [
            {
                name: np.asarray(out[i]).reshape(8, *out_avals[i].shape)[c]
                for i, name in enumerate(out_names)
            }
            for c in range(8)
        ]

    _CACHE["runner"] = run
    return run


def make_in_maps(foreground, mask):
    """Per-core host-side input prep (permute so own half is first)."""
    bs, ch, h, w = foreground.shape
    hw = h * w
    half = hw // 2
    f = np.ascontiguousarray(foreground.reshape(bs, ch, hw), dtype=np.float32)
    m = np.ascontiguousarray(mask.reshape(bs, hw), dtype=np.float32)
    in_maps = []
    for b in range(bs):
        k = f[b] + np.float32(1e-7)                 # [ch, hw], reference's +1e-7
        rstd = 1.0 / np.sqrt((k * k).sum(axis=0, dtype=np.float64))  # [hw]
        rstd = rstd.astype(np.float32)
        f16 = f[b].astype(np.float16)               # [ch, hw]
        km16 = ((rstd * m[b])[:, None] * k.T).astype(np.float16)  # [hw, ch]
        for hh in range(2):
            if hh == 0:
                fc, kmc, rc = f16, km16, rstd
            else:  # swap the two column-halves so own half comes first
                fc = np.concatenate([f16[:, half:], f16[:, :half]], axis=1)
                kmc = np.concatenate([km16[half:], km16[:half]], axis=0)
                rc = np.concatenate([rstd[half:], rstd[:half]])
            in_maps.append({
                "f16": np.ascontiguousarray(fc),
                "km16": np.ascontiguousarray(kmc),
                "rstd": np.ascontiguousarray(rc.reshape(hw // 128, 128).T),
            })
    return in_maps


def kernel(foreground, mask):
    foreground = np.asarray(foreground, dtype=np.float32)
    mask = np.asarray(mask, dtype=np.float32)
    bs, ch, h, w = foreground.shape
    hw = h * w

    in_maps = make_in_maps(foreground, mask)
    try:
        results = _get_runner()(in_maps)
    except Exception:
        # robust fallback: the generic SPMD entry point
        res = run_bass_kernel_spmd(_get_program(), in_maps, list(range(8)))
        results = res.results

    fmap = np.empty((bs, ch, h, w), dtype=np.float32)
    rows = h // 2
    for core in range(8):
        b, hh = core // 2, core % 2
        o = results[core]["o"]       # [ch, hw/2] unnormalized
        s = results[core]["s"]       # [1, hw/2] softmax denominator
        fmap[b, :, hh * rows:(hh + 1) * rows, :] = (o / s).reshape(ch, rows, w)

    mm = mask[:, 0:1]                    # [bs, 1, h, w]
    final = fmap * (1.0 - mm) + foreground * mm
    skip = mask.sum(axis=(1, 2, 3)) > (hw - 10)
    final[skip] = foreground[skip]
    return final.astype(np.float32)



# revision 2
# speedup vs baseline: 17.3589x; 17.3589x over previous
"""ContextualAttentionMask Trainium2 kernel (FP8 DoubleRow).

Math (per batch sample):
  f: [256, 4096] feature map (channels x pixels), m: [4096] mask
  K[j, :]    = f[:, j] + 1e-7          (per-pixel 1x1 kernel)
  rstd[j]    = 1 / ||K[j, :]||_2
  raw[j, n]  = sum_c f[c, j] * f[c, n]
  att[j, n]  = softmax_j(rstd[j] * raw[j, n])
  fmap[c, n] = sum_j rstd[j] * m[j] * K[j, c] * att[j, n]
  final      = fmap * (1 - m) + f * m  ;  skip branch if mask nearly all-ones

Device computes (per core, unnormalized; host divides, blends, skip-branch):
  E[j, n] = exp(rstd[j] * raw[j, n] + bias)   bias = 5 - max_j(1/rstd[j]),
                                              keeps E in (0, ~250] for e5m2
  o[c, n] = sum_j km8[j, c] * E[j, n]    km8 = e4m3(16 * rstd * m * K)
  s[n]    = sum_j E[j, n]

Both GEMMs run in fp8 with perf_mode=DoubleRow (contraction 256 per
matmul: 128 partitions x 2 interleave slots), which doubles PE FLOP
throughput vs fp16. The scores GEMM contracts the 256 channels in one
matmul; the accumulation GEMM contracts j in blocks of 256.

Sharding: 8 cores = 4 samples x 2 column-halves (2048 columns each).
Inputs host-permuted so each core's own half is columns 0..2047.
"""

import sys
from contextlib import ExitStack

import numpy as np

sys.path.insert(0, "/opt/trn_rl_repo")

from concourse import bacc, mybir, tile  # noqa: E402
from concourse.bass_utils import run_bass_kernel_spmd  # noqa: E402

FP32 = mybir.dt.float32
FP16 = mybir.dt.float16
FP8E4 = mybir.dt.float8e4
FP8E5 = mybir.dt.float8e5
DR = mybir.MatmulPerfMode.DoubleRow

CH = 256          # channels
J = 4096          # number of per-pixel kernels (= h*w)
NH = 2048         # columns handled per core (half of a sample)
QS = 512          # output column chunk width (PSUM bank)
BIAS_MARGIN = 5.0  # exp bias = BIAS_MARGIN - max_diag (host-computed)
KM_SCALE = 16.0   # host scales km by 16 into e4m3 sweet spot; host divides


def build_program(ch=CH, j_total=J, n_half=NH, loop_reps=1, skew=2,
                  bufs_sc=2, bufs_out=4, bufs_e=5, acc_fp32=0,
                  defer=3, psum_dma=0, n_acc=2, gps_copy=0):
    """Emit the per-core Bass/Tile program (SPMD across 8 cores).

    v3: each jb's two 512-col score matmuls land in one 2-bank PSUM tile
    [128, 1024]; a single 1024-wide exp and a single 1024-wide DVE add
    follow (halves the per-instruction bubbles on ACT and DVE). A group's
    sum-fold matmul + output DMAs are deferred `defer` jb-steps into the
    next group so the in-order PE queue never stalls on the tail DVE adds.
    psum_dma=1 DMAs o/s straight from PSUM (no DVE staging copy).
    skew is in jb steps.
    """
    n_jb = j_total // 128      # 32 j-blocks of 128
    n_t = j_total // 256       # 16 j-blocks of 256 (DoubleRow contraction)
    n_groups = n_half // (2 * QS)  # 2 column groups of 1024
    NQL = 2                    # 512-col chunks per group
    GW = NQL * QS              # group width (1024)
    ACC_DT = FP32 if acc_fp32 else FP16

    nc = bacc.Bacc("TRN2", target_bir_lowering=False, debug=False, num_devices=8)

    f8_d = nc.dram_tensor("f8", [128, 2 * j_total], FP8E4, kind="ExternalInput").ap()
    km_d = nc.dram_tensor("km8", [n_t * 128, 2 * ch], FP8E4, kind="ExternalInput").ap()
    rstd_d = nc.dram_tensor("rstd", [128, n_jb + 1], FP32, kind="ExternalInput").ap()
    o_d = nc.dram_tensor("o", [ch, n_half], FP32, kind="ExternalOutput").ap()
    s_d = nc.dram_tensor("s", [1, n_half], FP32, kind="ExternalOutput").ap()

    with tile.TileContext(nc) as tc, ExitStack() as ctx:
        const_p = ctx.enter_context(tc.tile_pool(name="const", bufs=1))
        f8_p = ctx.enter_context(tc.tile_pool(name="f8", bufs=1))
        km_p = ctx.enter_context(tc.tile_pool(name="km", bufs=n_t))
        e_p = ctx.enter_context(tc.tile_pool(name="e", bufs=bufs_e))
        acc_p = ctx.enter_context(tc.tile_pool(name="acc", bufs=4))
        osb_p = ctx.enter_context(tc.tile_pool(name="osb", bufs=4))
        ssb_p = ctx.enter_context(tc.tile_pool(name="ssb", bufs=2))
        ps_sc = ctx.enter_context(
            tc.tile_pool(name="ps_sc", bufs=bufs_sc, space="PSUM"))
        ps_out = ctx.enter_context(
            tc.tile_pool(name="ps_out", bufs=bufs_out, space="PSUM"))

        ones16 = const_p.tile([128, 1], ACC_DT)
        nc.vector.memset(ones16[:], 1.0)
        rb = const_p.tile([128, n_jb + 1], FP32, tag="rb")

        # fp8 feature map, [c%128, c//128, j] layout; chunked DMA so the
        # first matmuls start early. rstd+bias rides after the first chunk.
        f8t = f8_p.tile([128, 2, j_total], FP8E4, tag="f8t")
        bounds = [0, 512, 1024, 2048, j_total]
        for i, (a, b) in enumerate(zip(bounds[:-1], bounds[1:])):
            for sl in range(2):
                nc.sync.dma_start(
                    out=f8t[:, sl:sl + 1, a:b],
                    in_=f8_d[:, sl * j_total + a:sl * j_total + b],
                )
            if i == 0:
                nc.sync.dma_start(out=rb[:], in_=rstd_d[:, :])
            if i == 1:  # km needed once the accum matmuls begin
                pass
        km = []
        for t in range(n_t):
            kt = km_p.tile([128, 2, ch], FP8E4, tag="km", name=f"km{t}")
            nc.sync.dma_start(out=kt[:], in_=km_d[t * 128:(t + 1) * 128, :])
            km.append(kt)

        # fused main loop over column groups of 1024 (2 PSUM-chunk columns).
        # Per step jb: two DoubleRow scores matmuls (contraction = all 256
        # channels) into one 2-bank PSUM tile, one 1024-wide exp on ACT into
        # the e5m2 E tile, one 1024-wide DVE running sum. Accum matmuls for
        # a 256-j block trail by `skew` jb steps so the in-order PE queue
        # never waits on the ACT exp latency.
        def make_finalize(g, accs):
            def finalize():
                acc = accs[0]
                if len(accs) > 1:
                    nc.vector.tensor_add(acc[:], acc[:], accs[1][:])
                for ql in range(NQL):
                    nsl = slice(g * GW + ql * QS, g * GW + (ql + 1) * QS)
                    sum_ps = ps_sc.tile([1, QS], FP32, tag="sc", name="sum_ps")
                    nc.tensor.matmul(sum_ps[:], ones16[:],
                                     acc[:, ql * QS:(ql + 1) * QS],
                                     start=True, stop=True)
                    if psum_dma:
                        nc.sync.dma_start(out=s_d[0:1, nsl], in_=sum_ps[:])
                    else:
                        srow = ssb_p.tile([1, QS], FP32, tag="srow", name="srow")
                        nc.vector.tensor_copy(srow[:], sum_ps[:])
                        nc.sync.dma_start(out=s_d[0:1, nsl], in_=srow[:])
            return finalize

        pending = []
        for g in [gg for _ in range(loop_reps) for gg in range(n_groups)]:
            out_ps = [
                [ps_out.tile([128, QS], FP32, tag="out", name=f"out{cb}_{ql}")
                 for ql in range(NQL)]
                for cb in range(2)
            ]
            accs = [acc_p.tile([128, GW], ACC_DT, tag="acc", name=f"acc{a}")
                    for a in range(n_acc)]
            etiles = {}
            next_t = 0
            for s in range(n_jb + skew):
                if s == defer and pending:
                    pending.pop(0)()
                if s < n_jb:
                    jb = s
                    t, half = jb // 2, jb % 2
                    ps = ps_sc.tile([128, GW], FP32, tag="sc", name="ps")
                    for ql in range(NQL):
                        nsl = slice(g * GW + ql * QS, g * GW + (ql + 1) * QS)
                        nc.tensor.matmul(
                            ps[:, ql * QS:(ql + 1) * QS],
                            f8t[:, :, jb * 128:(jb + 1) * 128], f8t[:, :, nsl],
                            start=True, stop=True, perf_mode=DR,
                        )
                    if half == 0:
                        etiles[t] = e_p.tile([128, 2, GW], FP8E5,
                                             tag="e", name=f"e{t % 2}")
                    esl = etiles[t][:, half:half + 1, :]
                    nc.scalar.activation(
                        esl, ps[:], mybir.ActivationFunctionType.Exp,
                        bias=rb[:, n_jb:n_jb + 1], scale=rb[:, jb:jb + 1],
                    )
                    a = accs[jb % n_acc]
                    if jb < n_acc:
                        nc.vector.tensor_copy(a[:], esl)
                    else:
                        nc.vector.tensor_add(a[:], a[:], esl)
                # consume blocks whose produce finished `skew` jb steps ago
                while next_t < n_t and 2 * next_t + 1 + skew <= s:
                    t = next_t
                    for cb in range(2):
                        for ql in range(NQL):
                            nc.tensor.matmul(
                                out_ps[cb][ql][:],
                                km[t][:, :, cb * 128:(cb + 1) * 128],
                                etiles[t][:, :, ql * QS:(ql + 1) * QS],
                                start=(t == 0), stop=(t == n_t - 1),
                                perf_mode=DR,
                            )
                    etiles.pop(t)
                    next_t += 1

            # o outputs leave as soon as the accumulation groups close
            for cb in range(2):
                for ql in range(NQL):
                    nsl = slice(g * GW + ql * QS, g * GW + (ql + 1) * QS)
                    if psum_dma:
                        nc.sync.dma_start(
                            out=o_d[cb * 128:(cb + 1) * 128, nsl],
                            in_=out_ps[cb][ql][:])
                    else:
                        osb = osb_p.tile([128, QS], FP32, tag="osb", name="osb")
                        eng = nc.gpsimd if gps_copy else nc.vector
                        eng.tensor_copy(osb[:], out_ps[cb][ql][:])
                        nc.sync.dma_start(
                            out=o_d[cb * 128:(cb + 1) * 128, nsl], in_=osb[:])
            # the sum-fold matmul waits on the tail DVE adds; run it a few
            # steps into the next group so the PE queue keeps streaming
            pending.append(make_finalize(g, accs))
        for fin in pending:
            fin()

    nc.compile()
    return nc


_CACHE = {}


def _get_program():
    if "nc" not in _CACHE:
        _CACHE["nc"] = build_program()
    return _CACHE["nc"]


def _get_runner():
    """Cached sharded executable over 8 cores (same program/plugin as
    run_bass_kernel_spmd's axon path, but without per-call retracing)."""
    if "runner" in _CACHE:
        return _CACHE["runner"]
    import jax
    from jax.sharding import Mesh, NamedSharding, PartitionSpec
    from jax.experimental.shard_map import shard_map
    from concourse import bass2jax, mybir
    from concourse.bass2jax import _bass_exec_p, partition_id_tensor

    nc = _get_program()
    bass2jax.install_neuronx_cc_hook()
    pname = nc.partition_id_tensor.name if nc.partition_id_tensor else None

    in_names, out_names, out_avals = [], [], []
    for alloc in nc.m.functions[0].allocations:
        if not isinstance(alloc, mybir.MemoryLocationSet):
            continue
        name = alloc.memorylocations[0].name
        if alloc.kind == "ExternalInput":
            if name != pname:
                in_names.append(name)
        elif alloc.kind == "ExternalOutput":
            out_names.append(name)
            out_avals.append(
                jax.core.ShapedArray(
                    tuple(alloc.tensor_shape), mybir.dt.np(alloc.dtype)
                )
            )
    n_params, n_outs = len(in_names), len(out_names)
    all_in = in_names + out_names + ([pname] if pname else [])

    def _body(*args):
        operands = list(args)
        if pname is not None:
            operands.append(partition_id_tensor())
        return tuple(_bass_exec_p.bind(
            *operands, out_avals=tuple(out_avals), in_names=tuple(all_in),
            out_names=tuple(out_names), lowering_input_output_aliases=(),
            sim_require_finite=True, sim_require_nnan=True, nc=nc,
        ))

    devices = jax.devices()[:8]
    mesh = Mesh(np.asarray(devices), ("core",))
    spec = NamedSharding(mesh, PartitionSpec("core"))
    fn = jax.jit(
        shard_map(
            _body, mesh=mesh,
            in_specs=(PartitionSpec("core"),) * (n_params + n_outs),
            out_specs=(PartitionSpec("core"),) * n_outs,
            check_rep=False,
        ),
        donate_argnums=tuple(range(n_params, n_params + n_outs)),
        keep_unused=True,
    )
    zero_host = [
        np.zeros((8 * a.shape[0], *a.shape[1:]), a.dtype) for a in out_avals
    ]

    def run(in_maps):
        concat_in = [
            np.concatenate([np.asarray(m[name]) for m in in_maps], axis=0)
            for name in in_names
        ]
        zeros = [jax.device_put(z, spec) for z in zero_host]
        out = fn(*concat_in, *zeros)
        return [
            {
                name: np.asarray(out[i]).reshape(8, *out_avals[i].shape)[c]
                for i, name in enumerate(out_names)
            }
            for c in range(8)
        ]

    _CACHE["runner"] = run
    return run


def make_in_maps(foreground, mask):
    """Per-core host-side input prep (permute so own half is first)."""
    import ml_dtypes
    E4 = ml_dtypes.float8_e4m3

    bs, ch, h, w = foreground.shape
    hw = h * w
    half = hw // 2
    n_t = hw // 256
    f = np.ascontiguousarray(foreground.reshape(bs, ch, hw), dtype=np.float32)
    m = np.ascontiguousarray(mask.reshape(bs, hw), dtype=np.float32)
    in_maps = []
    for b in range(bs):
        k = f[b] + np.float32(1e-7)                 # [ch, hw], reference's +1e-7
        rstd = 1.0 / np.sqrt((k * k).sum(axis=0, dtype=np.float64))  # [hw]
        rstd = rstd.astype(np.float32)
        bias = np.float32(BIAS_MARGIN - (1.0 / rstd).max())
        f8 = f[b].astype(E4)                        # [ch, hw]
        km8 = ((KM_SCALE * rstd * m[b])[:, None] * k.T).astype(E4)  # [hw, ch]
        for hh in range(2):
            if hh == 0:
                fc, kmc, rc = f8, km8, rstd
            else:  # swap the two column-halves so own half comes first
                fc = np.concatenate([f8[:, half:], f8[:, :half]], axis=1)
                kmc = np.concatenate([km8[half:], km8[:half]], axis=0)
                rc = np.concatenate([rstd[half:], rstd[:half]])
            # f8 dram layout: [c%128, (c//128)*hw + j]
            f8_dram = np.concatenate([fc[:128], fc[128:]], axis=1)
            # km dram layout: rows = t*128 + (j%256)%128, cols = (j%256)//128*ch + c
            km_dram = (
                kmc.reshape(n_t, 2, 128, ch)
                .transpose(0, 2, 1, 3)
                .reshape(n_t * 128, 2 * ch)
            )
            rmat = rc.reshape(hw // 128, 128).T      # [128, n_jb]
            rb = np.concatenate(
                [rmat, np.full((128, 1), bias, np.float32)], axis=1)
            in_maps.append({
                "f8": np.ascontiguousarray(f8_dram),
                "km8": np.ascontiguousarray(km_dram),
                "rstd": np.ascontiguousarray(rb),
            })
    return in_maps


def kernel(foreground, mask):
    foreground = np.asarray(foreground, dtype=np.float32)
    mask = np.asarray(mask, dtype=np.float32)
    bs, ch, h, w = foreground.shape
    hw = h * w

    in_maps = make_in_maps(foreground, mask)
    try:
        results = _get_runner()(in_maps)
    except Exception:
        # robust fallback: the generic SPMD entry point
        res = run_bass_kernel_spmd(_get_program(), in_maps, list(range(8)))
        results = res.results

    fmap = np.empty((bs, ch, h, w), dtype=np.float32)
    rows = h // 2
    for core in range(8):
        b, hh = core // 2, core % 2
        o = results[core]["o"]       # [ch, hw/2] unnormalized, x KM_SCALE
        s = results[core]["s"]       # [1, hw/2] softmax denominator
        fmap[b, :, hh * rows:(hh + 1) * rows, :] = (
            o / (np.float32(KM_SCALE) * s)).reshape(ch, rows, w)

    mm = mask[:, 0:1]                    # [bs, 1, h, w]
    final = fmap * (1.0 - mm) + foreground * mm
    skip = mask.sum(axis=(1, 2, 3)) > (hw - 10)
    final[skip] = foreground[skip]
    return final.astype(np.float32)


# revision 4
# speedup vs baseline: 18.1795x; 1.0473x over previous
"""ContextualAttentionMask Trainium2 kernel (FP8 DoubleRow).

Math (per batch sample):
  f: [256, 4096] feature map (channels x pixels), m: [4096] mask
  K[j, :]    = f[:, j] + 1e-7          (per-pixel 1x1 kernel)
  rstd[j]    = 1 / ||K[j, :]||_2
  raw[j, n]  = sum_c f[c, j] * f[c, n]
  att[j, n]  = softmax_j(rstd[j] * raw[j, n])
  fmap[c, n] = sum_j rstd[j] * m[j] * K[j, c] * att[j, n]
  final      = fmap * (1 - m) + f * m  ;  skip branch if mask nearly all-ones

Device computes (per core, unnormalized; host divides, blends, skip-branch):
  E[j, n] = exp(rstd[j] * raw[j, n] + bias)   bias = 5 - max_j(1/rstd[j]),
                                              keeps E in (0, ~250] for e5m2
  o[c, n] = sum_j km8[j, c] * E[j, n]    km8 = e4m3(16 * rstd * m * K)
  s[n]    = sum_j E[j, n]

Both GEMMs run in fp8 with perf_mode=DoubleRow (contraction 256 per
matmul: 128 partitions x 2 interleave slots), which doubles PE FLOP
throughput vs fp16. The scores GEMM contracts the 256 channels in one
matmul; the accumulation GEMM contracts j in blocks of 256.

Sharding: 8 cores = 4 samples x 2 column-halves (2048 columns each).
Inputs host-permuted so each core's own half is columns 0..2047.
"""

import sys
from contextlib import ExitStack

import numpy as np

sys.path.insert(0, "/opt/trn_rl_repo")

from concourse import bacc, mybir, tile  # noqa: E402
from concourse.bass_utils import run_bass_kernel_spmd  # noqa: E402

FP32 = mybir.dt.float32
FP16 = mybir.dt.float16
FP8E4 = mybir.dt.float8e4
FP8E5 = mybir.dt.float8e5
DR = mybir.MatmulPerfMode.DoubleRow

CH = 256          # channels
J = 4096          # number of per-pixel kernels (= h*w)
NH = 2048         # columns handled per core (half of a sample)
QS = 512          # output column chunk width (PSUM bank)
BIAS_MARGIN = 5.0  # exp bias = BIAS_MARGIN - max_diag (host-computed)
KM_SCALE = 16.0   # host scales km by 16 into e4m3 sweet spot; host divides


def build_program(ch=CH, j_total=J, n_half=NH, loop_reps=1, skew=2,
                  bufs_sc=2, bufs_out=4, bufs_e=8, acc_fp32=0,
                  defer=3, psum_dma=0, n_acc=2, gps_copy=0, no_acc=0):
    """Emit the per-core Bass/Tile program (SPMD across 8 cores).

    v3: each jb's two 512-col score matmuls land in one 2-bank PSUM tile
    [128, 1024]; a single 1024-wide exp and a single 1024-wide DVE add
    follow (halves the per-instruction bubbles on ACT and DVE). A group's
    sum-fold matmul + output DMAs are deferred `defer` jb-steps into the
    next group so the in-order PE queue never stalls on the tail DVE adds.
    psum_dma=1 DMAs o/s straight from PSUM (no DVE staging copy).
    skew is in jb steps.
    """
    n_jb = j_total // 128      # 32 j-blocks of 128
    n_t = j_total // 256       # 16 j-blocks of 256 (DoubleRow contraction)
    n_groups = n_half // (2 * QS)  # 2 column groups of 1024
    NQL = 2                    # 512-col chunks per group
    GW = NQL * QS              # group width (1024)
    ACC_DT = FP32 if acc_fp32 else FP16

    nc = bacc.Bacc("TRN2", target_bir_lowering=False, debug=False, num_devices=8)

    f8_d = nc.dram_tensor("f8", [128, 2 * j_total], FP8E4, kind="ExternalInput").ap()
    km_d = nc.dram_tensor("km8", [n_t * 128, 2 * ch], FP8E4, kind="ExternalInput").ap()
    rstd_d = nc.dram_tensor("rstd", [128, n_jb + 1], FP32, kind="ExternalInput").ap()
    o_d = nc.dram_tensor("o", [ch, n_half], FP32, kind="ExternalOutput").ap()
    s_d = nc.dram_tensor("s", [1, n_half], FP32, kind="ExternalOutput").ap()

    with tile.TileContext(nc) as tc, ExitStack() as ctx:
        const_p = ctx.enter_context(tc.tile_pool(name="const", bufs=1))
        f8_p = ctx.enter_context(tc.tile_pool(name="f8", bufs=1))
        km_p = ctx.enter_context(tc.tile_pool(name="km", bufs=n_t))
        e_p = ctx.enter_context(tc.tile_pool(name="e", bufs=bufs_e))
        acc_p = ctx.enter_context(tc.tile_pool(name="acc", bufs=4))
        osb_p = ctx.enter_context(tc.tile_pool(name="osb", bufs=4))
        ssb_p = ctx.enter_context(tc.tile_pool(name="ssb", bufs=2))
        ps_sc = ctx.enter_context(
            tc.tile_pool(name="ps_sc", bufs=bufs_sc, space="PSUM"))
        ps_out = ctx.enter_context(
            tc.tile_pool(name="ps_out", bufs=bufs_out, space="PSUM"))

        ones16 = const_p.tile([128, 1], ACC_DT)
        nc.vector.memset(ones16[:], 1.0)
        rb = const_p.tile([128, n_jb + 1], FP32, tag="rb")

        # fp8 feature map, [c%128, c//128, j] layout; chunked DMA so the
        # first matmuls start early. rstd+bias rides after the first chunk.
        f8t = f8_p.tile([128, 2, j_total], FP8E4, tag="f8t")
        bounds = [0, 512, 1024, 2048, j_total]
        for i, (a, b) in enumerate(zip(bounds[:-1], bounds[1:])):
            for sl in range(2):
                nc.sync.dma_start(
                    out=f8t[:, sl:sl + 1, a:b],
                    in_=f8_d[:, sl * j_total + a:sl * j_total + b],
                )
            if i == 0:
                nc.sync.dma_start(out=rb[:], in_=rstd_d[:, :])
            if i == 1:  # km needed once the accum matmuls begin
                pass
        km = []
        for t in range(n_t):
            kt = km_p.tile([128, 2, ch], FP8E4, tag="km", name=f"km{t}")
            nc.sync.dma_start(out=kt[:], in_=km_d[t * 128:(t + 1) * 128, :])
            km.append(kt)

        # fused main loop over column groups of 1024 (2 PSUM-chunk columns).
        # Per step jb: two DoubleRow scores matmuls (contraction = all 256
        # channels) into one 2-bank PSUM tile, one 1024-wide exp on ACT into
        # the e5m2 E tile, one 1024-wide DVE running sum. Accum matmuls for
        # a 256-j block trail by `skew` jb steps so the in-order PE queue
        # never waits on the ACT exp latency.
        def make_finalize(g, accs):
            def finalize():
                if no_acc:  # probe: s never computed/stored
                    return
                acc = accs[0]
                if len(accs) > 1:
                    nc.vector.tensor_add(acc[:], acc[:], accs[1][:])
                # acc is [128, 2 * GW]: slot-0 partial | slot-1 partial.
                # Fold both 128-partition halves of each 512-col chunk into
                # one PSUM accumulation group (2 matmuls -> [1, QS]).
                for ql in range(NQL):
                    nsl = slice(g * GW + ql * QS, g * GW + (ql + 1) * QS)
                    sum_ps = ps_sc.tile([1, QS], FP32, tag="sc", name="sum_ps")
                    for sl in range(2):
                        nc.tensor.matmul(
                            sum_ps[:], ones16[:],
                            acc[:, sl * GW + ql * QS:sl * GW + (ql + 1) * QS],
                            start=(sl == 0), stop=(sl == 1))
                    if psum_dma:
                        nc.sync.dma_start(out=s_d[0:1, nsl], in_=sum_ps[:])
                    else:
                        srow = ssb_p.tile([1, QS], FP32, tag="srow", name="srow")
                        nc.vector.tensor_copy(srow[:], sum_ps[:])
                        nc.sync.dma_start(out=s_d[0:1, nsl], in_=srow[:])
            return finalize

        pending = []
        for g in [gg for _ in range(loop_reps) for gg in range(n_groups)]:
            out_ps = [
                [ps_out.tile([128, QS], FP32, tag="out", name=f"out{cb}_{ql}")
                 for ql in range(NQL)]
                for cb in range(2)
            ]
            accs = [acc_p.tile([128, 2 * GW], ACC_DT, tag="acc", name=f"acc{a}")
                    for a in range(n_acc)]
            etiles = {}
            next_t = 0
            for s in range(n_jb + skew):
                if s == defer and pending:
                    pending.pop(0)()
                if s < n_jb:
                    jb = s
                    t, half = jb // 2, jb % 2
                    ps = ps_sc.tile([128, GW], FP32, tag="sc", name="ps")
                    for ql in range(NQL):
                        nsl = slice(g * GW + ql * QS, g * GW + (ql + 1) * QS)
                        nc.tensor.matmul(
                            ps[:, ql * QS:(ql + 1) * QS],
                            f8t[:, :, jb * 128:(jb + 1) * 128], f8t[:, :, nsl],
                            start=True, stop=True, perf_mode=DR,
                        )
                    if half == 0:
                        etiles[t] = e_p.tile([128, 2, GW], FP8E5,
                                             tag="e", name=f"e{t % 2}")
                    esl = etiles[t][:, half:half + 1, :]
                    nc.scalar.activation(
                        esl, ps[:], mybir.ActivationFunctionType.Exp,
                        bias=rb[:, n_jb:n_jb + 1], scale=rb[:, jb:jb + 1],
                    )
                    # one wide DVE op per 256-j block (both slots at once)
                    if not no_acc and half == 1:
                        a = accs[t % n_acc]
                        if t < n_acc:
                            nc.vector.tensor_copy(a[:], etiles[t][:, :, :])
                        else:
                            nc.vector.tensor_add(a[:], a[:], etiles[t][:, :, :])
                # consume blocks whose produce finished `skew` jb steps ago
                while next_t < n_t and 2 * next_t + 1 + skew <= s:
                    t = next_t
                    for cb in range(2):
                        for ql in range(NQL):
                            nc.tensor.matmul(
                                out_ps[cb][ql][:],
                                km[t][:, :, cb * 128:(cb + 1) * 128],
                                etiles[t][:, :, ql * QS:(ql + 1) * QS],
                                start=(t == 0), stop=(t == n_t - 1),
                                perf_mode=DR,
                            )
                    etiles.pop(t)
                    next_t += 1

            # o outputs leave as soon as the accumulation groups close
            for cb in range(2):
                for ql in range(NQL):
                    nsl = slice(g * GW + ql * QS, g * GW + (ql + 1) * QS)
                    if psum_dma:
                        nc.sync.dma_start(
                            out=o_d[cb * 128:(cb + 1) * 128, nsl],
                            in_=out_ps[cb][ql][:])
                    else:
                        osb = osb_p.tile([128, QS], FP32, tag="osb", name="osb")
                        eng = nc.gpsimd if gps_copy else nc.vector
                        eng.tensor_copy(osb[:], out_ps[cb][ql][:])
                        nc.sync.dma_start(
                            out=o_d[cb * 128:(cb + 1) * 128, nsl], in_=osb[:])
            # the sum-fold matmul waits on the tail DVE adds; run it a few
            # steps into the next group so the PE queue keeps streaming
            pending.append(make_finalize(g, accs))
        for fin in pending:
            fin()

    nc.compile()
    return nc


_CACHE = {}


def _get_program():
    if "nc" not in _CACHE:
        _CACHE["nc"] = build_program()
    return _CACHE["nc"]


def _get_runner():
    """Cached sharded executable over 8 cores (same program/plugin as
    run_bass_kernel_spmd's axon path, but without per-call retracing)."""
    if "runner" in _CACHE:
        return _CACHE["runner"]
    import jax
    from jax.sharding import Mesh, NamedSharding, PartitionSpec
    from jax.experimental.shard_map import shard_map
    from concourse import bass2jax, mybir
    from concourse.bass2jax import _bass_exec_p, partition_id_tensor

    nc = _get_program()
    bass2jax.install_neuronx_cc_hook()
    pname = nc.partition_id_tensor.name if nc.partition_id_tensor else None

    in_names, out_names, out_avals = [], [], []
    for alloc in nc.m.functions[0].allocations:
        if not isinstance(alloc, mybir.MemoryLocationSet):
            continue
        name = alloc.memorylocations[0].name
        if alloc.kind == "ExternalInput":
            if name != pname:
                in_names.append(name)
        elif alloc.kind == "ExternalOutput":
            out_names.append(name)
            out_avals.append(
                jax.core.ShapedArray(
                    tuple(alloc.tensor_shape), mybir.dt.np(alloc.dtype)
                )
            )
    n_params, n_outs = len(in_names), len(out_names)
    all_in = in_names + out_names + ([pname] if pname else [])

    def _body(*args):
        operands = list(args)
        if pname is not None:
            operands.append(partition_id_tensor())
        return tuple(_bass_exec_p.bind(
            *operands, out_avals=tuple(out_avals), in_names=tuple(all_in),
            out_names=tuple(out_names), lowering_input_output_aliases=(),
            sim_require_finite=True, sim_require_nnan=True, nc=nc,
        ))

    devices = jax.devices()[:8]
    mesh = Mesh(np.asarray(devices), ("core",))
    spec = NamedSharding(mesh, PartitionSpec("core"))
    fn = jax.jit(
        shard_map(
            _body, mesh=mesh,
            in_specs=(PartitionSpec("core"),) * (n_params + n_outs),
            out_specs=(PartitionSpec("core"),) * n_outs,
            check_rep=False,
        ),
        donate_argnums=tuple(range(n_params, n_params + n_outs)),
        keep_unused=True,
    )
    zero_host = [
        np.zeros((8 * a.shape[0], *a.shape[1:]), a.dtype) for a in out_avals
    ]

    def run(in_maps):
        concat_in = [
            np.concatenate([np.asarray(m[name]) for m in in_maps], axis=0)
            for name in in_names
        ]
        zeros = [jax.device_put(z, spec) for z in zero_host]
        out = fn(*concat_in, *zeros)
        return [
            {
                name: np.asarray(out[i]).reshape(8, *out_avals[i].shape)[c]
                for i, name in enumerate(out_names)
            }
            for c in range(8)
        ]

    _CACHE["runner"] = run
    return run


def make_in_maps(foreground, mask):
    """Per-core host-side input prep (permute so own half is first)."""
    import ml_dtypes
    E4 = ml_dtypes.float8_e4m3

    bs, ch, h, w = foreground.shape
    hw = h * w
    half = hw // 2
    n_t = hw // 256
    f = np.ascontiguousarray(foreground.reshape(bs, ch, hw), dtype=np.float32)
    m = np.ascontiguousarray(mask.reshape(bs, hw), dtype=np.float32)
    in_maps = []
    for b in range(bs):
        k = f[b] + np.float32(1e-7)                 # [ch, hw], reference's +1e-7
        rstd = 1.0 / np.sqrt((k * k).sum(axis=0, dtype=np.float64))  # [hw]
        rstd = rstd.astype(np.float32)
        bias = np.float32(BIAS_MARGIN - (1.0 / rstd).max())
        f8 = f[b].astype(E4)                        # [ch, hw]
        km8 = ((KM_SCALE * rstd * m[b])[:, None] * k.T).astype(E4)  # [hw, ch]
        for hh in range(2):
            if hh == 0:
                fc, kmc, rc = f8, km8, rstd
            else:  # swap the two column-halves so own half comes first
                fc = np.concatenate([f8[:, half:], f8[:, :half]], axis=1)
                kmc = np.concatenate([km8[half:], km8[:half]], axis=0)
                rc = np.concatenate([rstd[half:], rstd[:half]])
            # f8 dram layout: [c%128, (c//128)*hw + j]
            f8_dram = np.concatenate([fc[:128], fc[128:]], axis=1)
            # km dram layout: rows = t*128 + (j%256)%128, cols = (j%256)//128*ch + c
            km_dram = (
                kmc.reshape(n_t, 2, 128, ch)
                .transpose(0, 2, 1, 3)
                .reshape(n_t * 128, 2 * ch)
            )
            rmat = rc.reshape(hw // 128, 128).T      # [128, n_jb]
            rb = np.concatenate(
                [rmat, np.full((128, 1), bias, np.float32)], axis=1)
            in_maps.append({
                "f8": np.ascontiguousarray(f8_dram),
                "km8": np.ascontiguousarray(km_dram),
                "rstd": np.ascontiguousarray(rb),
            })
    return in_maps


def kernel(foreground, mask):
    foreground = np.asarray(foreground, dtype=np.float32)
    mask = np.asarray(mask, dtype=np.float32)
    bs, ch, h, w = foreground.shape
    hw = h * w

    in_maps = make_in_maps(foreground, mask)
    try:
        results = _get_runner()(in_maps)
    except Exception:
        # robust fallback: the generic SPMD entry point
        res = run_bass_kernel_spmd(_get_program(), in_maps, list(range(8)))
        results = res.results

    fmap = np.empty((bs, ch, h, w), dtype=np.float32)
    rows = h // 2
    for core in range(8):
        b, hh = core // 2, core % 2
        o = results[core]["o"]       # [ch, hw/2] unnormalized, x KM_SCALE
        s = results[core]["s"]       # [1, hw/2] softmax denominator
        fmap[b, :, hh * rows:(hh + 1) * rows, :] = (
            o / (np.float32(KM_SCALE) * s)).reshape(ch, rows, w)

    mm = mask[:, 0:1]                    # [bs, 1, h, w]
    final = fmap * (1.0 - mm) + foreground * mm
    skip = mask.sum(axis=(1, 2, 3)) > (hw - 10)
    final[skip] = foreground[skip]
    return final.astype(np.float32)
